# revision 37
# baseline (speedup 1.0000x reference)
"""Distributed CAP-memory loss kernel for 8 TRN2 NeuronCores (fp8 edition).

Problem (see reference): given unit-norm features [B=256, D=2048] and a
memory bank [6, 2000, 2048], compute
  loss = sum_cam mean_cam(per-camera proxy CE)
       + 0.5 * sum_cam mean_cam(assoc loss over 6 positives + 50 hard negatives)

Distribution strategy (column/class sharding, interleaved):
  The 12000 memory rows are split so core k owns columns
  {j*2000 + k*250 + r : j in [0,6), r in [0,250)} -- an identical 250-wide
  slice of every camera block, so all 8 cores run the same SPMD program on
  1500 columns each.

Device program (per core):
  * inputs are pre-scaled x16 and quantized to fp8e4 on host (values stay
    well inside fp8e4's +-240 range); PSUM sims are 256x the true sims
  * sims = feats @ memT_local entirely in DoubleRow fp8 matmuls (2
    contraction rows per pass: ~2x the bf16 matmul rate at half the bf16
    DMA bytes).  memT is packed [P, kop*chunk, 2*512] so every
    (ko-pair, chunk) is one contiguous run and the DoubleRow [p, 2, n]
    moving AP has a 16B-aligned k-half stride.
  * epilogue per 500-col chunk: ACT reads PSUM once per camera block,
    writing exp(20*sim) to SBUF with the block's softmax denominator in
    the ACT accumulator; one DVE max8 extracts the chunk's top-8
    candidates (exp is monotone, the host takes log)
  * a few warm-up matmuls on dummy data keep the PE HAM un-throttled
    while the first DMA pieces land; mem/feats pieces alternate across
    the two HWDGE rings (sync/scalar) in PE-consumption order so
    consecutive pieces drain concurrently at full HBM rate.

The host merges the per-core stats ([256, 30] each): removes the
(fp8-simulated) positives from the candidate lists, takes the global
top-50 with an exactness certificate and an exact host fallback for the
few rows where 8-per-chunk was not enough, log-sum-exp combines,
segment sums -> scalar loss.
"""

import os
import sys
import types

import numpy as np

# ---------------------------------------------------------------- constants
B = 256          # batch
D = 2048         # feature dim
NCAMS = 6
C = 2000         # classes per camera
NG = NCAMS * C   # 12000 global columns
M = 8            # cores
W = C // M       # 250: per-core slice width inside each camera block
NL = NCAMS * W   # 1500 local columns per core
P = 128          # partitions
KO = D // P      # 16 contraction chunks
KOP = KO // 2    # 8 DoubleRow ko-pairs
BT = B // P      # 2 batch tiles
CHW = 500        # chunk width (= 2 camera blocks, fits one PSUM bank)
CHP = 528        # padded chunk width in packed memT (16B-aligned stride,
                 # NOT a multiple of 512 -- avoids SBUF bank conflicts on
                 # the DoubleRow k-half pair reads)
NCH = NL // CHW  # 3 chunks
NG3 = NCH * KOP  # 24 (chunk, kop) groups in packed memT (chunk-major)
CHUNK_OF_BLOCK = [0, 0, 1, 1, 2, 2]
BETA = 0.05
INV_BETA = 1.0 / BETA  # 20.0
BG_KNN = 50
NCAND = NCH * 8  # 24 candidates per core (top-8 per chunk)
SCALE = 16.0     # host-side fp8 pre-scale (exact power of 2)
SCALE2 = SCALE * SCALE  # sims in PSUM are SCALE2 x true sims
POS_TOL = 1e-4   # host-side positive-candidate matching tolerance (sims)
N_WARMUP_MM = 8  # dummy matmuls: ~3.4us of PE busy un-throttles the HAM
                 # (one full SHORT window) before the first real matmul
OUTC = NCAND + NCAMS  # 24 topk (exp domain) | 6 block sums (avg*250)

LAST_EXEC_NS = None
FALLBACK_COUNT = 0
_NC_CACHE = {}


def _install_axon_ntff_hook():
    """The agent image's antenv lacks axon_hooks; synthesize it so
    run_bass_kernel_spmd(trace=True) can capture NTFF profiles."""
    if "antenv.axon_hooks" in sys.modules:
        return
    mod = types.ModuleType("antenv.axon_hooks")
    state = {"hook": None}
    mod.set_axon_ntff_profile_hook = lambda h: state.__setitem__("hook", h)
    mod.get_axon_ntff_profile_hook = lambda: state["hook"]
    sys.modules["antenv.axon_hooks"] = mod
    try:
        import antenv

        antenv.axon_hooks = mod
    except Exception:
        pass
    try:
        from trn_agent_boot.trn_boot import _ntff_profile_via_ctypes

        hook = _ntff_profile_via_ctypes("/opt/axon/libaxon_pjrt.so")
        if hook is not None:
            mod.set_axon_ntff_profile_hook(hook)
    except Exception:
        pass


def build_nc():
    """Build + compile the single SPMD Bass program shared by all 8 cores."""
    import concourse.bacc as bacc
    import concourse.mybir as mybir
    import concourse.tile as tile

    f32 = mybir.dt.float32
    bf16 = mybir.dt.bfloat16
    f8 = mybir.dt.float8e4
    AF = mybir.ActivationFunctionType
    DR = mybir.MatmulPerfMode.DoubleRow

    nc = bacc.Bacc(
        "TRN2",
        target_bir_lowering=False,
        debug=False,
        enable_asserts=False,
        num_devices=M,
    )

    # featsT: [P, bt*kop, 2*128] (bt-major).  memT: [P, kop*chunk, 2*512].
    featsT_d = nc.dram_tensor("featsT", [P, BT * KOP, 2 * P], f8, kind="ExternalInput")
    memT_d = nc.dram_tensor("memT", [P, NG3, 2 * CHP], f8, kind="ExternalInput")
    out_d = nc.dram_tensor("out", [B, OUTC], f32, kind="ExternalOutput")

    with tile.TileContext(nc) as tc:
        with (
            tc.tile_pool(name="big", bufs=1) as big,
            tc.tile_pool(name="psum", bufs=1, space="PSUM") as psum,
        ):
            featsT_sb = big.tile([P, BT * KOP, 2 * P], f8, tag="feats")
            memT_sb = big.tile([P, NG3, 2 * CHP], f8, tag="mem")
            dummy = big.tile([P, 512], f8, tag="dummy")
            simsb = [
                big.tile([P, NL], f32, tag=f"simsb{b}", name=f"simsb{b}")
                for b in range(BT)
            ]
            outs = [
                big.tile([P, OUTC], f32, tag=f"outs{b}", name=f"outs{b}")
                for b in range(BT)
            ]
            warm_ps = psum.tile([P, 512], f32, tag="warm")
            ps = [
                [
                    psum.tile([P, CHW], f32, tag=f"ps{b}_{c}", name=f"ps{b}_{c}")
                    for c in range(NCH)
                ]
                for b in range(BT)
            ]

            # ---- warm-up matmuls (dummy data, result discarded) ----
            nc.vector.memset(dummy[:], 0)
            for _ in range(N_WARMUP_MM):
                nc.tensor.matmul(
                    warm_ps[:], dummy[:, 0:P], dummy[:], start=True, stop=True
                )

            # ---- streaming: alternate the two HWDGE rings, PE order ----
            # (each ring drains FIFO; alternating keeps arrival order close
            # to consumption order while two pieces drain concurrently)
            rings = [nc.sync, nc.scalar]

            def mem_piece(r, glo, ghi):
                rings[r].dma_start(
                    memT_sb[:, glo:ghi, :], memT_d[:, glo:ghi, :]
                )

            rings[0].dma_start(featsT_sb[:], featsT_d[:])
            mem_piece(1, 0, 2)       # c0 kp0-1
            mem_piece(0, 2, 4)       # c0 kp2-3
            mem_piece(1, 4, 6)       # c0 kp4-5
            mem_piece(0, 6, 8)       # c0 kp6-7
            mem_piece(1, 8, 12)      # c1 kp0-3
            mem_piece(0, 12, 16)     # c1 kp4-7
            mem_piece(1, 16, 20)     # c2 kp0-3
            mem_piece(0, 20, 24)     # c2 kp4-7

            def mm_dr(bt, kp, c):
                g = bt * KOP + kp
                lhsT = featsT_sb[:, g : g + 1, :].rearrange(
                    "p g (h c) -> p (g h) c", h=2
                )
                rhs = memT_sb[:, c * KOP + kp : c * KOP + kp + 1, :].rearrange(
                    "p g (h w) -> p (g h) w", h=2
                )[:, :, 0:CHW]
                nc.tensor.matmul(
                    ps[bt][c][:],
                    lhsT,
                    rhs,
                    start=(kp == 0),
                    stop=(kp == KOP - 1),
                    perf_mode=DR,
                )

            def epilogue(bt, c):
                # exp(20*sim) with per-camera-block f32 sums (ACT
                # accumulator); top-8 candidates via one DVE max8 over
                # the exp'd chunk (exp is monotone, the host takes log)
                for h in range(2):
                    j = 2 * c + h
                    nc.scalar.activation(
                        simsb[bt][:, j * W : (j + 1) * W],
                        ps[bt][c][:, h * W : (h + 1) * W],
                        AF.Exp,
                        scale=INV_BETA / SCALE2,
                        accum_out=outs[bt][:, NCAND + j : NCAND + j + 1],
                    )
                csl = slice(c * CHW, (c + 1) * CHW)
                nc.vector.max(
                    out=outs[bt][:, c * 8 : (c + 1) * 8], in_=simsb[bt][:, csl]
                )

            # chunk-outer: chunk c's epilogues overlap chunk c+1's matmuls
            for c in range(NCH):
                for bt in range(BT):
                    for kp in range(KOP):
                        mm_dr(bt, kp, c)
                    epilogue(bt, c)
            for bt in range(BT):
                rings[bt].dma_start(out_d[bt * P : (bt + 1) * P, :], outs[bt][:])

    nc.compile()
    return nc


def get_nc():
    if "nc" not in _NC_CACHE:
        _NC_CACHE["nc"] = build_nc()
    return _NC_CACHE["nc"]


def _f8():
    import ml_dtypes

    return np.dtype(ml_dtypes.float8_e4m3)


def shard_cols(k: int) -> np.ndarray:
    """Global memory-bank columns owned by core k."""
    return (
        np.arange(NCAMS)[:, None] * C + k * W + np.arange(W)[None, :]
    ).reshape(-1)


def pack_featsT(features: np.ndarray) -> np.ndarray:
    """[B, D] -> [P, BT*KOP, 2*128] fp8 (bt-major, then ko-pair)."""
    q = (features * SCALE).astype(_f8())
    # [D, B] -> [kop, h, p, bt, c] -> [p, bt, kop, h, c]
    arr = q.T.reshape(KOP, 2, P, BT, P).transpose(2, 3, 0, 1, 4)
    return np.ascontiguousarray(arr.reshape(P, BT * KOP, 2 * P))


def pack_memT(mem_flat_q: np.ndarray, cols: np.ndarray) -> np.ndarray:
    """fp8 [NG, D] -> [P, chunk*kop, 2*528] (chunk-major, cols padded)."""
    # [NL, D] -> [D, NL] -> [kop, h, p, chunk, w]
    arr = mem_flat_q[cols].T.reshape(KOP, 2, P, NCH, CHW)
    out = np.zeros((P, NCH, KOP, 2, CHP), _f8())
    out[:, :, :, :, :CHW] = arr.transpose(2, 3, 0, 1, 4)
    return np.ascontiguousarray(out.reshape(P, NG3, 2 * CHP))


def _loss_from_parts(pos_logits, lse_block, top50, cams):
    rows = np.arange(B)
    ce = lse_block[rows, cams] - pos_logits[rows, cams]
    logits = np.concatenate([pos_logits, INV_BETA * top50], axis=1)
    mx = logits.max(axis=1, keepdims=True)
    lse56 = mx[:, 0] + np.log(np.exp(logits - mx).sum(axis=1))
    assoc = lse56 - pos_logits.sum(axis=1) / NCAMS

    counts = np.bincount(cams, minlength=NCAMS).astype(np.float64)
    ce_sum = np.bincount(cams, weights=ce, minlength=NCAMS)
    as_sum = np.bincount(cams, weights=assoc, minlength=NCAMS)
    safe = np.maximum(counts, 1.0)
    present = counts > 0
    return np.sum(np.where(present, ce_sum / safe, 0.0)) + np.sum(
        np.where(present, 0.5 * as_sum / safe, 0.0)
    )


def host_combine(outs, features, memory, cams, labels, featsq, memq):
    """outs: [M, B, OUTC] device results.  featsq/memq: fp8-quantized
    (x16) inputs as f32, for simulating device positive values."""
    global FALLBACK_COUNT
    cand = outs[:, :, :NCAND].astype(np.float64)  # [M, B, 24] exp(20*sim)
    sexp = outs[:, :, NCAND:].astype(np.float64)  # [M, B, 6] block sums

    s_block = sexp.sum(axis=0)  # [B, 6] sum(exp(20*sims)) per camera
    lse_block = np.log(s_block)  # logsumexp of own-camera logits

    # positives: exact (f64) for the loss, fp8-simulated for matching
    feats64 = np.asarray(features, np.float64)
    pos_vals = np.einsum(
        "bd,jbd->bj",
        feats64,
        np.asarray(memory, np.float64)[:, labels, :],
        optimize=True,
    )  # [B, 6]
    pos_q = (
        np.einsum(
            "bd,jbd->bj",
            featsq.astype(np.float64),
            memq.reshape(NCAMS, C, D).astype(np.float64)[:, labels, :],
            optimize=True,
        )
        / SCALE2
    )

    # [B, M*NCH, 8] per-(core,chunk) candidate lists, back in sims domain
    percl = (
        cand.transpose(1, 0, 2)
        .reshape(B, M, NCH, 8)
        .reshape(B, M * NCH, 8)
    )
    percl = np.log(percl) / INV_BETA
    cmin_raw = percl.min(axis=2)  # pre-drop floor per (core,chunk)

    # Remove positives from the candidate lists.  Positive (i, j) can only
    # appear on core labels[i]//W, chunk CHUNK_OF_BLOCK[j]; drop the
    # closest value within POS_TOL (missing a true positive would corrupt
    # the hard negatives; over-dropping a near-equal genuine value is
    # harmless).
    own_core = labels // W  # [B]
    for j in range(NCAMS):
        cl = own_core * NCH + CHUNK_OF_BLOCK[j]  # [B] chunk-list index
        lists = percl[np.arange(B), cl]  # [B, 8] (fancy idx: copy)
        diff = np.abs(lists - pos_q[:, j : j + 1])
        am = diff.argmin(axis=1)
        hit = diff[np.arange(B), am] < POS_TOL
        lists[hit, am[hit]] = -np.inf
        percl[np.arange(B), cl] = lists

    flat = percl.reshape(B, -1)
    top50 = -np.partition(-flat, BG_KNN - 1, axis=1)[:, :BG_KNN]
    t50 = top50[:, BG_KNN - 1]  # [B] 50th largest of the union

    # Exactness certificate: every (core,chunk)'s smallest extracted
    # candidate must lie strictly below the union's 50th value, proving no
    # unseen value could reach the global top-50.
    bad = (cmin_raw >= t50[:, None]).any(axis=1)
    if bad.any():
        # Exact fallback for insufficient rows: recompute on the host.
        FALLBACK_COUNT += int(bad.sum())
        mem_flat = np.asarray(memory, np.float32).reshape(NG, D)
        idx = np.nonzero(bad)[0]
        sims = np.asarray(features, np.float32)[idx] @ mem_flat.T
        colsg = np.arange(NG)
        for p, i in enumerate(idx):
            row = sims[p].astype(np.float64)
            row[colsg % C == labels[i]] = -np.inf
            top50[i] = -np.sort(-row)[:BG_KNN]

    return np.float32(
        _loss_from_parts(INV_BETA * pos_vals, lse_block, top50, cams)
    )


def kernel(features, memory, cams, labels, trace: bool = None):
    global LAST_EXEC_NS
    _install_axon_ntff_hook()
    from concourse.bass_utils import run_bass_kernel_spmd

    features = np.asarray(features, dtype=np.float32)
    memory = np.asarray(memory, dtype=np.float32)
    cams = np.asarray(cams).astype(np.int64)
    labels = np.asarray(labels).astype(np.int64)

    nc = get_nc()

    mem_flat = memory.reshape(NG, D)
    memq8 = (mem_flat * SCALE).astype(_f8())
    featsT = pack_featsT(features)
    in_maps = [
        {"featsT": featsT, "memT": pack_memT(memq8, shard_cols(k))}
        for k in range(M)
    ]

    if trace is None:
        trace = os.environ.get("CAP_TRACE", "1") == "1"
    res = run_bass_kernel_spmd(
        nc, in_maps, core_ids=list(range(M)), trace=trace
    )
    if res.exec_time_ns is not None:
        LAST_EXEC_NS = res.exec_time_ns

    outs = np.stack([r["out"] for r in res.results])  # [M, B, OUTC]
    featsq = (features * SCALE).astype(_f8()).astype(np.float32)
    memq = memq8.astype(np.float32)
    return np.asarray(
        host_combine(outs, features, memory, cams, labels, featsq, memq),
        dtype=np.float32,
    )


# ------------------------------------------------------------------ helpers
def expected_core_out(features, memory, labels, k: int) -> np.ndarray:
    """Numpy model of what core k's device program should output [B, OUTC]."""
    import ml_dtypes

    f8 = np.dtype(ml_dtypes.float8_e4m3)
    fq = (np.asarray(features, np.float32) * SCALE).astype(f8).astype(np.float32)
    mem_flat = np.asarray(memory, np.float32).reshape(NG, D)
    mq = (mem_flat * SCALE).astype(f8).astype(np.float32)
    cols = shard_cols(k)
    sims = fq @ mq[cols].T  # [B, NL], scaled x256
    e = np.exp((INV_BETA / SCALE2) * sims).astype(np.float32)
    out = np.zeros((B, OUTC), np.float32)
    for j in range(NCAMS):
        jsl = slice(j * W, (j + 1) * W)
        out[:, NCAND + j] = e[:, jsl].astype(np.float64).sum(axis=1)
    for c in range(NCH):
        srt = -np.sort(-e[:, c * CHW : (c + 1) * CHW], axis=1)
        out[:, c * 8 : (c + 1) * 8] = srt[:, :8]
    return out


# revision 39
# speedup vs baseline: 1.2224x; 1.2224x over previous
"""Distributed CAP-memory loss kernel for 8 TRN2 NeuronCores (fp8 edition).

Problem (see reference): given unit-norm features [B=256, D=2048] and a
memory bank [6, 2000, 2048], compute
  loss = sum_cam mean_cam(per-camera proxy CE)
       + 0.5 * sum_cam mean_cam(assoc loss over 6 positives + 50 hard negatives)

Distribution strategy (column/class sharding, interleaved):
  The 12000 memory rows are split so core k owns columns
  {j*2000 + k*250 + r : j in [0,6), r in [0,250)} -- an identical 250-wide
  slice of every camera block, so all 8 cores run the same SPMD program on
  1500 columns each.

Device program (per core):
  * inputs are pre-scaled x16 and quantized to fp8e4 on host (values stay
    well inside fp8e4's +-240 range); PSUM sims are 256x the true sims
  * sims = feats @ memT_local entirely in DoubleRow fp8 matmuls (2
    contraction rows per pass: ~2x the bf16 matmul rate at half the bf16
    DMA bytes).  memT is packed [P, kop*chunk, 2*512] so every
    (ko-pair, chunk) is one contiguous run and the DoubleRow [p, 2, n]
    moving AP has a 16B-aligned k-half stride.
  * epilogue per 500-col chunk: ACT reads PSUM once per camera block,
    writing exp(20*sim) to SBUF with the block's softmax denominator in
    the ACT accumulator; one DVE max8 extracts the chunk's top-8
    candidates (exp is monotone, the host takes log)
  * a few warm-up matmuls on dummy data keep the PE HAM un-throttled
    while the first DMA pieces land; mem/feats pieces alternate across
    the two HWDGE rings (sync/scalar) in PE-consumption order so
    consecutive pieces drain concurrently at full HBM rate.

The host merges the per-core stats ([256, 30] each): removes the
(fp8-simulated) positives from the candidate lists, takes the global
top-50 with an exactness certificate and an exact host fallback for the
few rows where 8-per-chunk was not enough, log-sum-exp combines,
segment sums -> scalar loss.
"""

import os
import sys
import types

import numpy as np

# ---------------------------------------------------------------- constants
B = 256          # batch
D = 2048         # feature dim
NCAMS = 6
C = 2000         # classes per camera
NG = NCAMS * C   # 12000 global columns
M = 8            # cores
W = C // M       # 250: per-core slice width inside each camera block
NL = NCAMS * W   # 1500 local columns per core
P = 128          # partitions
KO = D // P      # 16 contraction chunks
KOP = KO // 2    # 8 DoubleRow ko-pairs
BT = B // P      # 2 batch tiles
CHW = 500        # chunk width (= 2 camera blocks, fits one PSUM bank)
CHP = 528        # padded chunk width in packed memT (16B-aligned stride,
                 # NOT a multiple of 512 -- avoids SBUF bank conflicts on
                 # the DoubleRow k-half pair reads)
NCH = NL // CHW  # 3 chunks
NG3 = NCH * KOP  # 24 (chunk, kop) groups in packed memT (chunk-major)
CHUNK_OF_BLOCK = [0, 0, 1, 1, 2, 2]
BETA = 0.05
INV_BETA = 1.0 / BETA  # 20.0
BG_KNN = 50
NCAND = NCH * 8  # 24 candidates per core (top-8 per chunk)
SCALE = 16.0     # host-side fp8 pre-scale (exact power of 2)
SCALE2 = SCALE * SCALE  # sims in PSUM are SCALE2 x true sims
POS_TOL = 1e-4   # host-side positive-candidate matching tolerance (sims)
N_WARMUP_MM = 8  # dummy matmuls: ~3.4us of PE busy un-throttles the HAM
                 # (one full SHORT window) before the first real matmul
OUTC = NCAND + NCAMS  # 24 topk (exp domain) | 6 block sums (avg*250)

LAST_EXEC_NS = None
FALLBACK_COUNT = 0
_NC_CACHE = {}


def _install_axon_ntff_hook():
    """The agent image's antenv lacks axon_hooks; synthesize it so
    run_bass_kernel_spmd(trace=True) can capture NTFF profiles."""
    if "antenv.axon_hooks" in sys.modules:
        return
    mod = types.ModuleType("antenv.axon_hooks")
    state = {"hook": None}
    mod.set_axon_ntff_profile_hook = lambda h: state.__setitem__("hook", h)
    mod.get_axon_ntff_profile_hook = lambda: state["hook"]
    sys.modules["antenv.axon_hooks"] = mod
    try:
        import antenv

        antenv.axon_hooks = mod
    except Exception:
        pass
    try:
        from trn_agent_boot.trn_boot import _ntff_profile_via_ctypes

        hook = _ntff_profile_via_ctypes("/opt/axon/libaxon_pjrt.so")
        if hook is not None:
            mod.set_axon_ntff_profile_hook(hook)
    except Exception:
        pass


def build_nc():
    """Build + compile the single SPMD Bass program shared by all 8 cores."""
    import concourse.bacc as bacc
    import concourse.mybir as mybir
    import concourse.tile as tile

    f32 = mybir.dt.float32
    bf16 = mybir.dt.bfloat16
    f8 = mybir.dt.float8e4
    AF = mybir.ActivationFunctionType
    DR = mybir.MatmulPerfMode.DoubleRow

    nc = bacc.Bacc(
        "TRN2",
        target_bir_lowering=False,
        debug=False,
        enable_asserts=False,
        num_devices=M,
    )

    # featsT: [P, bt*kop, 2*128] (bt-major).  memT: [P, kop*chunk, 2*512].
    featsT_d = nc.dram_tensor("featsT", [P, BT * KOP, 2 * P], f8, kind="ExternalInput")
    memT_d = nc.dram_tensor("memT", [P, NG3, 2 * CHP], f8, kind="ExternalInput")
    out_d = nc.dram_tensor("out", [B, OUTC], f32, kind="ExternalOutput")

    with tile.TileContext(nc) as tc:
        with (
            tc.tile_pool(name="big", bufs=1) as big,
            tc.tile_pool(name="psum", bufs=1, space="PSUM") as psum,
        ):
            featsT_sb = big.tile([P, BT * KOP, 2 * P], f8, tag="feats")
            memT_sb = big.tile([P, NG3, 2 * CHP], f8, tag="mem")
            dummy = big.tile([P, 512], f8, tag="dummy")
            simsb = [
                big.tile([P, NL], f32, tag=f"simsb{b}", name=f"simsb{b}")
                for b in range(BT)
            ]
            outs = [
                big.tile([P, OUTC], f32, tag=f"outs{b}", name=f"outs{b}")
                for b in range(BT)
            ]
            warm_ps = psum.tile([P, 512], f32, tag="warm")
            ps = [
                [
                    psum.tile([P, CHW], f32, tag=f"ps{b}_{c}", name=f"ps{b}_{c}")
                    for c in range(NCH)
                ]
                for b in range(BT)
            ]

            # ---- warm-up matmuls (dummy data, result discarded) ----
            nc.vector.memset(dummy[:], 0)
            for _ in range(N_WARMUP_MM):
                nc.tensor.matmul(
                    warm_ps[:], dummy[:, 0:P], dummy[:], start=True, stop=True
                )

            # ---- streaming: alternate the two HWDGE rings, PE order ----
            # (each ring drains FIFO; alternating keeps arrival order close
            # to consumption order while two pieces drain concurrently)
            rings = [nc.sync, nc.scalar]

            def mem_piece(r, glo, ghi):
                rings[r].dma_start(
                    memT_sb[:, glo:ghi, :], memT_d[:, glo:ghi, :]
                )

            rings[0].dma_start(featsT_sb[:], featsT_d[:])
            mem_piece(1, 0, 2)       # c0 kp0-1
            mem_piece(0, 2, 4)       # c0 kp2-3
            mem_piece(1, 4, 6)       # c0 kp4-5
            mem_piece(0, 6, 8)       # c0 kp6-7
            mem_piece(1, 8, 12)      # c1 kp0-3
            mem_piece(0, 12, 16)     # c1 kp4-7
            mem_piece(1, 16, 20)     # c2 kp0-3
            mem_piece(0, 20, 22)     # c2 kp4-5
            mem_piece(1, 22, 24)     # c2 kp6-7 (small: its completion
                                     # receipt gates only the last 4 MMs)

            def mm_dr(bt, kp, c):
                g = bt * KOP + kp
                lhsT = featsT_sb[:, g : g + 1, :].rearrange(
                    "p g (h c) -> p (g h) c", h=2
                )
                rhs = memT_sb[:, c * KOP + kp : c * KOP + kp + 1, :].rearrange(
                    "p g (h w) -> p (g h) w", h=2
                )[:, :, 0:CHW]
                nc.tensor.matmul(
                    ps[bt][c][:],
                    lhsT,
                    rhs,
                    start=(kp == 0),
                    stop=(kp == KOP - 1),
                    perf_mode=DR,
                )

            def epilogue(bt, c):
                # exp(20*sim) with per-camera-block f32 sums (ACT
                # accumulator); top-8 candidates via one DVE max8 over
                # the exp'd chunk (exp is monotone, the host takes log)
                for h in range(2):
                    j = 2 * c + h
                    nc.scalar.activation(
                        simsb[bt][:, j * W : (j + 1) * W],
                        ps[bt][c][:, h * W : (h + 1) * W],
                        AF.Exp,
                        scale=INV_BETA / SCALE2,
                        accum_out=outs[bt][:, NCAND + j : NCAND + j + 1],
                    )
                csl = slice(c * CHW, (c + 1) * CHW)
                nc.vector.max(
                    out=outs[bt][:, c * 8 : (c + 1) * 8], in_=simsb[bt][:, csl]
                )

            # chunk-outer: chunk c's epilogues overlap chunk c+1's matmuls.
            # Early chunks run kp-outer/bt-inner so each mem piece feeds
            # back-to-back matmul pairs (halves the DMA lookahead needed
            # while the stream is still priming); the last chunk runs
            # bt-outer so bt0's epilogue overlaps bt1's matmuls and the
            # kernel tail is one bt's epilogue chain.
            for c in range(NCH - 1):
                for kp in range(KOP):
                    for bt in range(BT):
                        mm_dr(bt, kp, c)
                for bt in range(BT):
                    epilogue(bt, c)
            for bt in range(BT):
                for kp in range(KOP):
                    mm_dr(bt, kp, NCH - 1)
                epilogue(bt, NCH - 1)
            for bt in range(BT):
                rings[bt].dma_start(out_d[bt * P : (bt + 1) * P, :], outs[bt][:])

    nc.compile()
    return nc


def get_nc():
    if "nc" not in _NC_CACHE:
        _NC_CACHE["nc"] = build_nc()
    return _NC_CACHE["nc"]


def _f8():
    import ml_dtypes

    return np.dtype(ml_dtypes.float8_e4m3)


def shard_cols(k: int) -> np.ndarray:
    """Global memory-bank columns owned by core k."""
    return (
        np.arange(NCAMS)[:, None] * C + k * W + np.arange(W)[None, :]
    ).reshape(-1)


def pack_featsT(features: np.ndarray) -> np.ndarray:
    """[B, D] -> [P, BT*KOP, 2*128] fp8 (bt-major, then ko-pair)."""
    q = (features * SCALE).astype(_f8())
    # [D, B] -> [kop, h, p, bt, c] -> [p, bt, kop, h, c]
    arr = q.T.reshape(KOP, 2, P, BT, P).transpose(2, 3, 0, 1, 4)
    return np.ascontiguousarray(arr.reshape(P, BT * KOP, 2 * P))


def pack_memT(mem_flat_q: np.ndarray, cols: np.ndarray) -> np.ndarray:
    """fp8 [NG, D] -> [P, chunk*kop, 2*528] (chunk-major, cols padded)."""
    # [NL, D] -> [D, NL] -> [kop, h, p, chunk, w]
    arr = mem_flat_q[cols].T.reshape(KOP, 2, P, NCH, CHW)
    out = np.zeros((P, NCH, KOP, 2, CHP), _f8())
    out[:, :, :, :, :CHW] = arr.transpose(2, 3, 0, 1, 4)
    return np.ascontiguousarray(out.reshape(P, NG3, 2 * CHP))


def _loss_from_parts(pos_logits, lse_block, top50, cams):
    rows = np.arange(B)
    ce = lse_block[rows, cams] - pos_logits[rows, cams]
    logits = np.concatenate([pos_logits, INV_BETA * top50], axis=1)
    mx = logits.max(axis=1, keepdims=True)
    lse56 = mx[:, 0] + np.log(np.exp(logits - mx).sum(axis=1))
    assoc = lse56 - pos_logits.sum(axis=1) / NCAMS

    counts = np.bincount(cams, minlength=NCAMS).astype(np.float64)
    ce_sum = np.bincount(cams, weights=ce, minlength=NCAMS)
    as_sum = np.bincount(cams, weights=assoc, minlength=NCAMS)
    safe = np.maximum(counts, 1.0)
    present = counts > 0
    return np.sum(np.where(present, ce_sum / safe, 0.0)) + np.sum(
        np.where(present, 0.5 * as_sum / safe, 0.0)
    )


def host_combine(outs, features, memory, cams, labels, featsq, memq):
    """outs: [M, B, OUTC] device results.  featsq/memq: fp8-quantized
    (x16) inputs as f32, for simulating device positive values."""
    global FALLBACK_COUNT
    cand = outs[:, :, :NCAND].astype(np.float64)  # [M, B, 24] exp(20*sim)
    sexp = outs[:, :, NCAND:].astype(np.float64)  # [M, B, 6] block sums

    s_block = sexp.sum(axis=0)  # [B, 6] sum(exp(20*sims)) per camera
    lse_block = np.log(s_block)  # logsumexp of own-camera logits

    # positives: exact (f64) for the loss, fp8-simulated for matching
    feats64 = np.asarray(features, np.float64)
    pos_vals = np.einsum(
        "bd,jbd->bj",
        feats64,
        np.asarray(memory, np.float64)[:, labels, :],
        optimize=True,
    )  # [B, 6]
    pos_q = (
        np.einsum(
            "bd,jbd->bj",
            featsq.astype(np.float64),
            memq.reshape(NCAMS, C, D).astype(np.float64)[:, labels, :],
            optimize=True,
        )
        / SCALE2
    )

    # [B, M*NCH, 8] per-(core,chunk) candidate lists, back in sims domain
    percl = (
        cand.transpose(1, 0, 2)
        .reshape(B, M, NCH, 8)
        .reshape(B, M * NCH, 8)
    )
    percl = np.log(percl) / INV_BETA
    cmin_raw = percl.min(axis=2)  # pre-drop floor per (core,chunk)

    # Remove positives from the candidate lists.  Positive (i, j) can only
    # appear on core labels[i]//W, chunk CHUNK_OF_BLOCK[j]; drop the
    # closest value within POS_TOL (missing a true positive would corrupt
    # the hard negatives; over-dropping a near-equal genuine value is
    # harmless).
    own_core = labels // W  # [B]
    for j in range(NCAMS):
        cl = own_core * NCH + CHUNK_OF_BLOCK[j]  # [B] chunk-list index
        lists = percl[np.arange(B), cl]  # [B, 8] (fancy idx: copy)
        diff = np.abs(lists - pos_q[:, j : j + 1])
        am = diff.argmin(axis=1)
        hit = diff[np.arange(B), am] < POS_TOL
        lists[hit, am[hit]] = -np.inf
        percl[np.arange(B), cl] = lists

    flat = percl.reshape(B, -1)
    top50 = -np.partition(-flat, BG_KNN - 1, axis=1)[:, :BG_KNN]
    t50 = top50[:, BG_KNN - 1]  # [B] 50th largest of the union

    # Exactness certificate: every (core,chunk)'s smallest extracted
    # candidate must lie strictly below the union's 50th value, proving no
    # unseen value could reach the global top-50.
    bad = (cmin_raw >= t50[:, None]).any(axis=1)
    if bad.any():
        # Exact fallback for insufficient rows: recompute on the host.
        FALLBACK_COUNT += int(bad.sum())
        mem_flat = np.asarray(memory, np.float32).reshape(NG, D)
        idx = np.nonzero(bad)[0]
        sims = np.asarray(features, np.float32)[idx] @ mem_flat.T
        colsg = np.arange(NG)
        for p, i in enumerate(idx):
            row = sims[p].astype(np.float64)
            row[colsg % C == labels[i]] = -np.inf
            top50[i] = -np.sort(-row)[:BG_KNN]

    return np.float32(
        _loss_from_parts(INV_BETA * pos_vals, lse_block, top50, cams)
    )


def kernel(features, memory, cams, labels, trace: bool = None):
    global LAST_EXEC_NS
    _install_axon_ntff_hook()
    from concourse.bass_utils import run_bass_kernel_spmd

    features = np.asarray(features, dtype=np.float32)
    memory = np.asarray(memory, dtype=np.float32)
    cams = np.asarray(cams).astype(np.int64)
    labels = np.asarray(labels).astype(np.int64)

    nc = get_nc()

    mem_flat = memory.reshape(NG, D)
    memq8 = (mem_flat * SCALE).astype(_f8())
    featsT = pack_featsT(features)
    in_maps = [
        {"featsT": featsT, "memT": pack_memT(memq8, shard_cols(k))}
        for k in range(M)
    ]

    if trace is None:
        trace = os.environ.get("CAP_TRACE", "1") == "1"
    res = run_bass_kernel_spmd(
        nc, in_maps, core_ids=list(range(M)), trace=trace
    )
    if res.exec_time_ns is not None:
        LAST_EXEC_NS = res.exec_time_ns

    outs = np.stack([r["out"] for r in res.results])  # [M, B, OUTC]
    featsq = (features * SCALE).astype(_f8()).astype(np.float32)
    memq = memq8.astype(np.float32)
    return np.asarray(
        host_combine(outs, features, memory, cams, labels, featsq, memq),
        dtype=np.float32,
    )


# ------------------------------------------------------------------ helpers
def expected_core_out(features, memory, labels, k: int) -> np.ndarray:
    """Numpy model of what core k's device program should output [B, OUTC]."""
    import ml_dtypes

    f8 = np.dtype(ml_dtypes.float8_e4m3)
    fq = (np.asarray(features, np.float32) * SCALE).astype(f8).astype(np.float32)
    mem_flat = np.asarray(memory, np.float32).reshape(NG, D)
    mq = (mem_flat * SCALE).astype(f8).astype(np.float32)
    cols = shard_cols(k)
    sims = fq @ mq[cols].T  # [B, NL], scaled x256
    e = np.exp((INV_BETA / SCALE2) * sims).astype(np.float32)
    out = np.zeros((B, OUTC), np.float32)
    for j in range(NCAMS):
        jsl = slice(j * W, (j + 1) * W)
        out[:, NCAND + j] = e[:, jsl].astype(np.float64).sum(axis=1)
    for c in range(NCH):
        srt = -np.sort(-e[:, c * CHW : (c + 1) * CHW], axis=1)
        out[:, c * 8 : (c + 1) * 8] = srt[:, :8]
    return out
